# revision 10
# baseline (speedup 1.0000x reference)
"""Deformable cross-attention Trainium2 kernel.

Problem (hardcoded): N=32768 queries, M=32768 kv tokens, C=256, H=8 heads,
P=4 points, 1-D bilinear sampling along the token axis.

Sharding: queries split across 8 NeuronCores (4096/core); key/value tables and
weights replicated. No collectives.

Per-core plan:
  1. loc phase (transposed orientation, [hp=32 partitions x q free]):
     x = (sigmoid(q@Wr_x + br_x) + (q@Wo_x + bo_x)) * (M-1)
     x0 = clamp(round_rne(x-0.5), 0, M-1)   (== floor for lerp purposes)
     wx = x - x0;  A = (1-wx)/sqrt(32);  B = wx/sqrt(32)
     x0 -> int16, dumped to DRAM for gather-index replication.
  2. vproj phase: v = value@Wv + bv per 128-row tile (fp32 matmuls with PE
     transposes), then build the combined per-head table
     ckv[h][m] = [v[m, 32h:32h+32] | key[m, 32h:32h+32]]  (64 f32 = 256B rows)
  3. gather phase: per (q-chunk of 1024, head): one dma_gather op with
     num_idxs=4096 (q x P taps), elem_step=64, elem_size=128 -> each tap
     returns [v0|k0|v1|k1] for rows x0, x0+1. Pad row M duplicates row M-1 so
     the clipped x1 case is exact.
  4. scores s = (q.k0)*A + (q.k1)*B, softmax over P, out += a0*v0 + a1*v1,
     then output projection out@Wout + bout (fp32).
"""
import numpy as np
from contextlib import ExitStack

import concourse.bass as bass
import concourse.tile as tile
from concourse import mybir, bacc
from concourse.bass_utils import run_bass_kernel_spmd
from concourse.masks import make_identity
from concourse.vector_clock import ScopedClock

F32 = mybir.dt.float32
I16 = mybir.dt.int16
I32 = mybir.dt.int32
AF = mybir.ActivationFunctionType
OP = mybir.AluOpType

NC_CORES = 8
N, M, C, H, P = 32768, 32768, 256, 8, 4
CH = C // H                      # 32
NL = N // NC_CORES               # 4096 queries per core
NT = NL // 128                   # 32 q-tiles
QCHUNK = 1024                    # queries per gather chunk
NCHUNK = NL // QCHUNK            # 4
NIDX = QCHUNK * P                # 4096 taps per gather op
QS = QCHUNK // 128               # 8 q-subtiles per chunk
R32 = float(1.0 / np.sqrt(np.float32(CH)))
SCALE = float(M - 1)

# ---------------------------------------------------------------------------
# walrus in this toolchain rejects >1 sem wait per instruction; redistribute
# excess waits onto single-wait no-ops, and do the same for the TileContext
# tail drain.

def _patched_drain_and_barrier(self, tick_clock, wait_clock):
    nc = self.nc
    tmp = nc.sync.nop(nofuse=True)
    wait_clock.add_sem_waits(tmp.ins, ScopedClock({None: tick_clock.global_clock}))
    si = tmp.ins.sync_info
    if si is not None and si.on_wait is not None and len(si.on_wait) > 1:
        waits = list(si.on_wait)
        ups = list(si.on_update or [])
        tmp.ins.sync_info = mybir.SyncInfo(on_wait=waits[:1], on_update=ups)
        for w in waits[1:]:
            n = nc.sync.nop(nofuse=True)
            n.ins.sync_info = mybir.SyncInfo(on_wait=[w], on_update=[])
    nc.sync.drain()
    nc.all_engine_barrier()
    assert self.sems is not None
    popped = nc._tile_sem_poison_stack.pop()
    assert popped is self._sem_poison
    nc.clear_and_free_semaphores(list(self.sems.allocated().values()))
    nc.all_engine_barrier()


def _split_excess_waits(nc, limit=1):
    k = 0
    for f in nc.m.functions:
        for b in f.blocks:
            insts = b.instructions
            out = []
            for ins in insts:
                si = ins.sync_info
                waits = list(si.on_wait) if si is not None and si.on_wait else []
                if len(waits) > limit:
                    rest, keep = waits[:-limit], waits[-limit:]
                    for i in range(0, len(rest), limit):
                        nop = mybir.InstNoOp(name=f"wsplit-{k}", ins=[], outs=[])
                        k += 1
                        nop.engine = ins.engine
                        nop.sync_info = mybir.SyncInfo(
                            on_wait=rest[i:i + limit], on_update=[])
                        try:
                            nc.register_instruction(nop, overwrite=True)
                        except Exception:
                            pass
                        out.append(nop)
                    ins.sync_info = mybir.SyncInfo(
                        on_wait=keep, on_update=list(si.on_update or []))
                out.append(ins)
            if len(out) != len(insts):
                b.instructions = out


tile.TileContext._drain_and_barrier = _patched_drain_and_barrier

# ---------------------------------------------------------------------------


def _bcast(ap, axis, n):
    """Insert a [0, n] broadcast dim at position `axis` of an AP."""
    dims = [list(d) for d in ap.ap]
    dims.insert(axis, [0, n])
    return bass.AP(tensor=ap.tensor, offset=ap.offset, ap=dims)

def build_program():
    nc = bacc.Bacc("TRN2", target_bir_lowering=False, debug=False,
                   num_devices=NC_CORES)

    qs_t = nc.declare_dram_parameter("qs", [NL, C], F32, isOutput=False)
    key_t = nc.declare_dram_parameter("key", [M, C], F32, isOutput=False)
    val_t = nc.declare_dram_parameter("value", [M, C], F32, isOutput=False)
    wx_t = nc.declare_dram_parameter("Wx", [C, 64], F32, isOutput=False)
    bx_t = nc.declare_dram_parameter("bx", [64, 1], F32, isOutput=False)
    wv_t = nc.declare_dram_parameter("Wv", [C, C], F32, isOutput=False)
    bv_t = nc.declare_dram_parameter("bv", [C], F32, isOutput=False)
    wo_t = nc.declare_dram_parameter("Wout", [C, C], F32, isOutput=False)
    bo_t = nc.declare_dram_parameter("bout", [C], F32, isOutput=False)
    out_t = nc.declare_dram_parameter("out", [NL, C], F32, isOutput=True)

    ckv = nc.dram_tensor("ckv", [H, M + 2, 2 * CH], F32)
    x0d = nc.dram_tensor("x0d", [H, NCHUNK, 16, P * QCHUNK // 16], I16)

    with tile.TileContext(nc) as tc:
        with ExitStack() as ctx:
            consts = ctx.enter_context(tc.tile_pool(name="consts", bufs=1))
            locp = ctx.enter_context(tc.tile_pool(name="locp", bufs=3))
            vp = ctx.enter_context(tc.tile_pool(name="vp", bufs=4))
            mp = ctx.enter_context(tc.tile_pool(name="mp", bufs=2))
            smp = ctx.enter_context(tc.tile_pool(name="smp", bufs=2))
            gp_pool = ctx.enter_context(tc.tile_pool(name="gp", bufs=3))
            ps_t = ctx.enter_context(
                tc.tile_pool(name="ps_t", bufs=3, space="PSUM"))
            ps_m = ctx.enter_context(
                tc.tile_pool(name="ps_m", bufs=2, space="PSUM"))

            # ---- constants ----
            ident = consts.tile([128, 128], F32)
            make_identity(nc, ident[:])
            wx_sb = consts.tile([128, 2, 64], F32)
            nc.sync.dma_start(out=wx_sb[:],
                              in_=bass.AP(tensor=wx_t, offset=0,
                                          ap=[[64, 128], [8192, 2], [1, 64]]))
            bx_sb = consts.tile([64, 1], F32)
            nc.sync.dma_start(out=bx_sb[:], in_=bx_t[:])
            wv_sb = consts.tile([128, 2, 256], F32)
            nc.sync.dma_start(out=wv_sb[:],
                              in_=bass.AP(tensor=wv_t, offset=0,
                                          ap=[[256, 128], [32768, 2], [1, 256]]))
            wo_sb = consts.tile([128, 2, 256], F32)
            nc.sync.dma_start(out=wo_sb[:],
                              in_=bass.AP(tensor=wo_t, offset=0,
                                          ap=[[256, 128], [32768, 2], [1, 256]]))
            bv_b = consts.tile([128, 256], F32)
            nc.sync.dma_start(out=bv_b[:],
                              in_=bass.AP(tensor=bv_t, offset=0,
                                          ap=[[0, 128], [1, 256]]))
            bo_b = consts.tile([128, 256], F32)
            nc.sync.dma_start(out=bo_b[:],
                              in_=bass.AP(tensor=bo_t, offset=0,
                                          ap=[[0, 128], [1, 256]]))
            qn = consts.tile([128, NT, 256], F32)
            nc.sync.dma_start(out=qn[:],
                              in_=bass.AP(tensor=qs_t, offset=0,
                                          ap=[[256, 128], [32768, NT], [1, 256]]))
            x0i = consts.tile([H * P, NL], I16)
            a_res = consts.tile([128, NT, H * P], F32)
            b_res = consts.tile([128, NT, H * P], F32)

            # ---- loc phase ----
            for t in range(NT):
                qT = locp.tile([128, 2, 128], F32, tag="qT")
                for c in range(2):
                    pt = ps_t.tile([128, 128], F32, tag="pt")
                    nc.tensor.transpose(out=pt[:],
                                        in_=qn[:, t, 128 * c:128 * (c + 1)],
                                        identity=ident[:])
                    nc.scalar.copy(out=qT[:, c, :], in_=pt[:])
                pl = ps_m.tile([64, 128], F32, tag="pm")
                for c in range(2):
                    nc.tensor.matmul(out=pl[:], lhsT=wx_sb[:, c, :], rhs=qT[:, c, :],
                                     start=(c == 0), stop=(c == 1))
                sg = locp.tile([32, 128], F32, tag="sg")
                nc.scalar.activation(out=sg[:], in_=pl[0:32, :], func=AF.Sigmoid,
                                     bias=bx_sb[0:32, 0:1])
                off = locp.tile([32, 128], F32, tag="off")
                nc.scalar.activation(out=off[:], in_=pl[32:64, :],
                                     func=AF.Identity, bias=bx_sb[32:64, 0:1])
                x = locp.tile([32, 128], F32, tag="x")
                nc.vector.tensor_add(out=x[:], in0=sg[:], in1=off[:])
                nc.vector.tensor_scalar_mul(out=x[:], in0=x[:], scalar1=SCALE)
                y = locp.tile([32, 128], F32, tag="y")
                nc.vector.tensor_scalar_sub(out=y[:], in0=x[:], scalar1=0.5)
                yi = locp.tile([32, 128], I32, tag="yi")
                nc.vector.tensor_copy(out=yi[:], in_=y[:])
                x0c = locp.tile([32, 128], F32, tag="x0c")
                nc.vector.tensor_copy(out=x0c[:], in_=yi[:])
                nc.vector.tensor_scalar(out=x0c[:], in0=x0c[:], scalar1=0.0,
                                        scalar2=float(M - 1), op0=OP.max,
                                        op1=OP.min)
                wxt = locp.tile([32, 128], F32, tag="wxt")
                nc.vector.tensor_sub(out=wxt[:], in0=x[:], in1=x0c[:])
                at = locp.tile([32, 128], F32, tag="at")
                nc.vector.tensor_scalar(out=at[:], in0=wxt[:], scalar1=1.0,
                                        scalar2=-R32, op0=OP.subtract,
                                        op1=OP.mult)
                bt = locp.tile([32, 128], F32, tag="bt")
                nc.vector.tensor_scalar_mul(out=bt[:], in0=wxt[:], scalar1=R32)
                nc.vector.tensor_copy(out=x0i[:, 128 * t:128 * (t + 1)],
                                      in_=x0c[:])
                for src, dst in ((at, a_res), (bt, b_res)):
                    pt2 = ps_t.tile([128, 32], F32, tag="pt")
                    nc.tensor.transpose(out=pt2[:], in_=src[:],
                                        identity=ident[0:32, 0:32])
                    nc.vector.tensor_copy(out=dst[:, t, :], in_=pt2[:])
            for h in range(H):
                for ci in range(NCHUNK):
                    nc.scalar.dma_start(
                        out=bass.AP(tensor=x0d,
                                    offset=(h * NCHUNK + ci) * P * QCHUNK,
                                    ap=[[QCHUNK // 16, P], [1, QCHUNK // 16], [P * QCHUNK // 16, 16]]),
                        in_=x0i[4 * h:4 * (h + 1), ci * QCHUNK:(ci + 1) * QCHUNK])

            # ---- vproj + table build ----
            for mt in range(256):
                vtile = vp.tile([128, 256], F32, tag="vtile")
                nc.sync.dma_start(out=vtile[:], in_=val_t[128 * mt:128 * (mt + 1), :])
                ktile = vp.tile([128, 256], F32, tag="ktile")
                nc.scalar.dma_start(out=ktile[:], in_=key_t[128 * mt:128 * (mt + 1), :])
                vT = vp.tile([128, 2, 128], F32, tag="vT")
                for c in range(2):
                    pt = ps_t.tile([128, 128], F32, tag="pt")
                    nc.tensor.transpose(out=pt[:], in_=vtile[:, 128 * c:128 * (c + 1)],
                                        identity=ident[:])
                    nc.scalar.copy(out=vT[:, c, :], in_=pt[:])
                pv = ps_m.tile([128, 256], F32, tag="pm")
                for c in range(2):
                    nc.tensor.matmul(out=pv[:], lhsT=vT[:, c, :], rhs=wv_sb[:, c, :],
                                     start=(c == 0), stop=(c == 1))
                csb = vp.tile([128, H, 2 * CH], F32, tag="csb")
                nc.vector.tensor_add(
                    out=csb[:, :, 0:CH],
                    in0=pv[:].rearrange("p (h e) -> p h e", h=H),
                    in1=bv_b[:].rearrange("p (h e) -> p h e", h=H))
                nc.vector.tensor_copy(
                    out=csb[:, :, CH:2 * CH],
                    in_=ktile[:].rearrange("p (h e) -> p h e", h=H))
                eng = [nc.sync, nc.scalar]
                eng[mt % 2].dma_start(
                    out=bass.AP(tensor=ckv, offset=128 * mt * 2 * CH,
                                ap=[[2 * CH, 128], [(M + 2) * 2 * CH, H], [1, 2 * CH]]),
                    in_=csb[:])
                if mt == 255:
                    eng[1].dma_start(
                        out=bass.AP(tensor=ckv, offset=M * 2 * CH,
                                    ap=[[2 * CH, 1], [(M + 2) * 2 * CH, H], [1, 2 * CH]]),
                        in_=csb[127:128, :, :])

            # ---- main loop: gather + attention ----
            for ci in range(NCHUNK):
                oacc = mp.tile([128, QS, H, CH], F32, tag="oacc")
                for h in range(H):
                    idx = mp.tile([128, P * (QCHUNK // 16)], I16, tag="idx")
                    nc.sync.dma_start(
                        out=idx[:],
                        in_=bass.AP(tensor=x0d,
                                    offset=(h * NCHUNK + ci) * P * QCHUNK,
                                    ap=[[0, 8], [P * QCHUNK // 16, 16], [1, P * QCHUNK // 16]]))
                    g = gp_pool.tile([128, NIDX // 128, 2 * 2 * CH], F32, tag="g")
                    nc.gpsimd.dma_gather(
                        out_ap=g[:],
                        in_ap=bass.AP(tensor=ckv, offset=h * (M + 2) * 2 * CH,
                                      ap=[[2 * CH, M], [1, 4 * CH]]),
                        idxs_ap=idx[:],
                        num_idxs=NIDX,
                        num_idxs_reg=NIDX,
                        elem_size=4 * CH,
                        elem_step=2 * CH,
                        single_packet=False,
                    )
                    g4 = g[:].rearrange("p (a b) e -> p a b e", a=P)
                    qb = _bcast(qn[:, ci * QS:(ci + 1) * QS, CH * h:CH * (h + 1)], 1, P)
                    prod = smp.tile([128, P, QS, CH], F32, tag="prod")
                    s0 = smp.tile([128, P, QS], F32, tag="s0")
                    nc.vector.tensor_mul(out=prod[:], in0=g4[:, :, :, CH:2 * CH], in1=qb)
                    nc.vector.tensor_reduce(out=s0[:], in_=prod[:],
                                            axis=mybir.AxisListType.X, op=OP.add)
                    s1 = smp.tile([128, P, QS], F32, tag="s1")
                    nc.vector.tensor_mul(out=prod[:], in0=g4[:, :, :, 3 * CH:4 * CH], in1=qb)
                    nc.vector.tensor_reduce(out=s1[:], in_=prod[:],
                                            axis=mybir.AxisListType.X, op=OP.add)
                    asl = a_res[:, ci * QS:(ci + 1) * QS, P * h:P * (h + 1)] \
                        .rearrange("p a b -> p b a")
                    bsl = b_res[:, ci * QS:(ci + 1) * QS, P * h:P * (h + 1)] \
                        .rearrange("p a b -> p b a")
                    s = smp.tile([128, P, QS], F32, tag="s")
                    nc.vector.tensor_mul(out=s[:], in0=s0[:], in1=asl)
                    nc.vector.tensor_mul(out=s1[:], in0=s1[:], in1=bsl)
                    nc.vector.tensor_add(out=s[:], in0=s[:], in1=s1[:])
                    # softmax over P
                    mx = smp.tile([128, QS], F32, tag="mx")
                    m2 = smp.tile([128, QS], F32, tag="m2")
                    nc.vector.tensor_tensor(out=mx[:], in0=s[:, 0, :], in1=s[:, 1, :], op=OP.max)
                    nc.vector.tensor_tensor(out=m2[:], in0=s[:, 2, :], in1=s[:, 3, :], op=OP.max)
                    nc.vector.tensor_tensor(out=mx[:], in0=mx[:], in1=m2[:], op=OP.max)
                    mxb = _bcast(mx[:], 1, P)
                    nc.vector.tensor_sub(out=s[:], in0=s[:], in1=mxb)
                    e = smp.tile([128, P, QS], F32, tag="e")
                    nc.scalar.activation(out=e[:], in_=s[:], func=AF.Exp)
                    su = smp.tile([128, QS], F32, tag="su")
                    nc.vector.tensor_add(out=su[:], in0=e[:, 0, :], in1=e[:, 1, :])
                    nc.vector.tensor_add(out=m2[:], in0=e[:, 2, :], in1=e[:, 3, :])
                    nc.vector.tensor_add(out=su[:], in0=su[:], in1=m2[:])
                    rcp = smp.tile([128, QS], F32, tag="rcp")
                    nc.vector.reciprocal(out=rcp[:], in_=su[:])
                    nc.vector.tensor_scalar_mul(out=rcp[:], in0=rcp[:],
                                                scalar1=float(np.sqrt(np.float32(CH))))
                    attn = smp.tile([128, P, QS], F32, tag="attn")
                    rcb = _bcast(rcp[:], 1, P)
                    nc.vector.tensor_mul(out=attn[:], in0=e[:], in1=rcb)
                    a0 = smp.tile([128, P, QS], F32, tag="a0")
                    a1 = smp.tile([128, P, QS], F32, tag="a1")
                    nc.vector.tensor_mul(out=a0[:], in0=attn[:], in1=asl)
                    nc.vector.tensor_mul(out=a1[:], in0=attn[:], in1=bsl)
                    # weighted values
                    t3 = smp.tile([128, P, QS, CH], F32, tag="t3")
                    a0b = _bcast(a0[:], 3, CH)
                    a1b = _bcast(a1[:], 3, CH)
                    nc.vector.tensor_mul(out=t3[:], in0=g4[:, :, :, 0:CH], in1=a0b)
                    nc.vector.tensor_mul(out=prod[:], in0=g4[:, :, :, 2 * CH:3 * CH], in1=a1b)
                    nc.vector.tensor_add(out=t3[:], in0=t3[:], in1=prod[:])
                    u0 = smp.tile([128, QS, CH], F32, tag="u0")
                    nc.vector.tensor_add(out=u0[:], in0=t3[:, 0, :, :], in1=t3[:, 1, :, :])
                    u1 = smp.tile([128, QS, CH], F32, tag="u1")
                    nc.vector.tensor_add(out=u1[:], in0=t3[:, 2, :, :], in1=t3[:, 3, :, :])
                    nc.vector.tensor_add(out=oacc[:, :, h, :], in0=u0[:], in1=u1[:])
                # output projection for this chunk
                for q in range(QS):
                    rT = mp.tile([128, 2, 128], F32, tag="rT")
                    rsl = oacc[:, q, :, :].rearrange("p h e -> p (h e)")
                    for c in range(2):
                        pt = ps_t.tile([128, 128], F32, tag="pt")
                        nc.tensor.transpose(out=pt[:], in_=rsl[:, 128 * c:128 * (c + 1)],
                                            identity=ident[:])
                        nc.scalar.copy(out=rT[:, c, :], in_=pt[:])
                    po = ps_m.tile([128, 256], F32, tag="pm")
                    for c in range(2):
                        nc.tensor.matmul(out=po[:], lhsT=rT[:, c, :], rhs=wo_sb[:, c, :],
                                         start=(c == 0), stop=(c == 1))
                    osb = mp.tile([128, 256], F32, tag="osb")
                    nc.vector.tensor_add(out=osb[:], in0=po[:], in1=bo_b[:])
                    row0 = ci * QCHUNK + q * 128
                    nc.scalar.dma_start(out=out_t[row0:row0 + 128, :], in_=osb[:])

    nc.compile()
    _split_excess_waits(nc)
    return nc


_PROGRAM = None


def _get_program():
    global _PROGRAM
    if _PROGRAM is None:
        _PROGRAM = build_program()
    return _PROGRAM


def kernel(query, key, value, Wr, br, Wo, bo, Wv, bv, Wout, bout):
    query = np.ascontiguousarray(np.asarray(query, dtype=np.float32))
    key = np.ascontiguousarray(np.asarray(key, dtype=np.float32))
    value = np.ascontiguousarray(np.asarray(value, dtype=np.float32))
    Wr = np.asarray(Wr, dtype=np.float32)
    br = np.asarray(br, dtype=np.float32)
    Wo = np.asarray(Wo, dtype=np.float32)
    bo = np.asarray(bo, dtype=np.float32)
    Wv = np.ascontiguousarray(np.asarray(Wv, dtype=np.float32))
    bv = np.asarray(bv, dtype=np.float32)
    Wout = np.ascontiguousarray(np.asarray(Wout, dtype=np.float32))
    bout = np.asarray(bout, dtype=np.float32)

    Wx = np.ascontiguousarray(
        np.concatenate([Wr[:, 0::2], Wo[:, 0::2]], axis=1))
    bx = np.ascontiguousarray(
        np.concatenate([br[0::2], bo[0::2]])[:, None])

    nc = _get_program()
    in_maps = []
    for c in range(NC_CORES):
        in_maps.append({
            "qs": query[c * NL:(c + 1) * NL],
            "key": key,
            "value": value,
            "Wx": Wx,
            "bx": bx,
            "Wv": Wv,
            "bv": bv,
            "Wout": Wout,
            "bout": bout,
        })
    res = run_bass_kernel_spmd(nc, in_maps, list(range(NC_CORES)))
    out = np.concatenate([res.results[c]["out"] for c in range(NC_CORES)], axis=0)
    return out


# revision 11
# speedup vs baseline: 1.3288x; 1.3288x over previous
"""Deformable cross-attention Trainium2 kernel.

Problem (hardcoded): N=32768 queries, M=32768 kv tokens, C=256, H=8 heads,
P=4 points, 1-D bilinear sampling along the token axis.

Sharding: queries split across 8 NeuronCores (4096/core); key/value tables and
weights replicated. No collectives.

Per-core plan:
  1. loc phase (transposed orientation, [hp=32 partitions x q free]):
     x = (sigmoid(q@Wr_x + br_x) + (q@Wo_x + bo_x)) * (M-1)
     x0 = clamp(round_rne(x-0.5), 0, M-1)   (== floor for lerp purposes)
     wx = x - x0;  A = (1-wx)/sqrt(32);  B = wx/sqrt(32)
     x0 -> int16, dumped to DRAM for gather-index replication.
  2. vproj phase: v = value@Wv + bv per 128-row tile (fp32 matmuls with PE
     transposes), then build the combined per-head table
     ckv[h][m] = [v[m, 32h:32h+32] | key[m, 32h:32h+32]]  (64 f32 = 256B rows)
  3. gather phase: per (q-chunk of 1024, head): one dma_gather op with
     num_idxs=4096 (q x P taps), elem_step=64, elem_size=128 -> each tap
     returns [v0|k0|v1|k1] for rows x0, x0+1. Pad row M duplicates row M-1 so
     the clipped x1 case is exact.
  4. scores s = (q.k0)*A + (q.k1)*B, softmax over P, out += a0*v0 + a1*v1,
     then output projection out@Wout + bout (fp32).
"""
import numpy as np
from contextlib import ExitStack

import concourse.bass as bass
import concourse.tile as tile
from concourse import mybir, bacc
from concourse.bass_utils import run_bass_kernel_spmd
from concourse.masks import make_identity
from concourse.vector_clock import ScopedClock

F32 = mybir.dt.float32
I16 = mybir.dt.int16
I32 = mybir.dt.int32
AF = mybir.ActivationFunctionType
OP = mybir.AluOpType

NC_CORES = 8
N, M, C, H, P = 32768, 32768, 256, 8, 4
CH = C // H                      # 32
NL = N // NC_CORES               # 4096 queries per core
NT = NL // 128                   # 32 q-tiles
QCHUNK = 1024                    # queries per gather chunk
NCHUNK = NL // QCHUNK            # 4
NIDX = QCHUNK * P                # 4096 taps per gather op
QS = QCHUNK // 128               # 8 q-subtiles per chunk
R32 = float(1.0 / np.sqrt(np.float32(CH)))
SCALE = float(M - 1)

# ---------------------------------------------------------------------------
# walrus in this toolchain rejects >1 sem wait per instruction; redistribute
# excess waits onto single-wait no-ops, and do the same for the TileContext
# tail drain.

def _patched_drain_and_barrier(self, tick_clock, wait_clock):
    nc = self.nc
    tmp = nc.sync.nop(nofuse=True)
    wait_clock.add_sem_waits(tmp.ins, ScopedClock({None: tick_clock.global_clock}))
    si = tmp.ins.sync_info
    if si is not None and si.on_wait is not None and len(si.on_wait) > 1:
        waits = list(si.on_wait)
        ups = list(si.on_update or [])
        tmp.ins.sync_info = mybir.SyncInfo(on_wait=waits[:1], on_update=ups)
        for w in waits[1:]:
            n = nc.sync.nop(nofuse=True)
            n.ins.sync_info = mybir.SyncInfo(on_wait=[w], on_update=[])
    nc.sync.drain()
    nc.all_engine_barrier()
    assert self.sems is not None
    popped = nc._tile_sem_poison_stack.pop()
    assert popped is self._sem_poison
    nc.clear_and_free_semaphores(list(self.sems.allocated().values()))
    nc.all_engine_barrier()


def _split_excess_waits(nc, limit=1):
    k = 0
    for f in nc.m.functions:
        for b in f.blocks:
            insts = b.instructions
            out = []
            for ins in insts:
                si = ins.sync_info
                waits = list(si.on_wait) if si is not None and si.on_wait else []
                if len(waits) > limit:
                    rest, keep = waits[:-limit], waits[-limit:]
                    for i in range(0, len(rest), limit):
                        nop = mybir.InstNoOp(name=f"wsplit-{k}", ins=[], outs=[])
                        k += 1
                        nop.engine = ins.engine
                        nop.sync_info = mybir.SyncInfo(
                            on_wait=rest[i:i + limit], on_update=[])
                        try:
                            nc.register_instruction(nop, overwrite=True)
                        except Exception:
                            pass
                        out.append(nop)
                    ins.sync_info = mybir.SyncInfo(
                        on_wait=keep, on_update=list(si.on_update or []))
                out.append(ins)
            if len(out) != len(insts):
                b.instructions = out


tile.TileContext._drain_and_barrier = _patched_drain_and_barrier

# ---------------------------------------------------------------------------


def _bcast(ap, axis, n):
    """Insert a [0, n] broadcast dim at position `axis` of an AP."""
    dims = [list(d) for d in ap.ap]
    dims.insert(axis, [0, n])
    return bass.AP(tensor=ap.tensor, offset=ap.offset, ap=dims)

def build_program():
    nc = bacc.Bacc("TRN2", target_bir_lowering=False, debug=False,
                   num_devices=NC_CORES)

    qs_t = nc.declare_dram_parameter("qs", [NL, C], F32, isOutput=False)
    key_t = nc.declare_dram_parameter("key", [M, C], F32, isOutput=False)
    val_t = nc.declare_dram_parameter("value", [M, C], F32, isOutput=False)
    wx_t = nc.declare_dram_parameter("Wx", [C, 64], F32, isOutput=False)
    bx_t = nc.declare_dram_parameter("bx", [64, 1], F32, isOutput=False)
    wv_t = nc.declare_dram_parameter("Wv", [C, C], F32, isOutput=False)
    bv_t = nc.declare_dram_parameter("bv", [C], F32, isOutput=False)
    wo_t = nc.declare_dram_parameter("Wout", [C, C], F32, isOutput=False)
    bo_t = nc.declare_dram_parameter("bout", [C], F32, isOutput=False)
    out_t = nc.declare_dram_parameter("out", [NL, C], F32, isOutput=True)

    ckv = nc.dram_tensor("ckv", [H, M + 2, 2 * CH], F32)
    x0d = nc.dram_tensor("x0d", [H, NCHUNK, 16, P * QCHUNK // 16], I16)

    with tile.TileContext(nc) as tc:
        with ExitStack() as ctx:
            consts = ctx.enter_context(tc.tile_pool(name="consts", bufs=1))
            locp = ctx.enter_context(tc.tile_pool(name="locp", bufs=2))
            vp = ctx.enter_context(tc.tile_pool(name="vp", bufs=5))
            mp = ctx.enter_context(tc.tile_pool(name="mp", bufs=3))
            smp = ctx.enter_context(tc.tile_pool(name="smp", bufs=2))
            ps_t = ctx.enter_context(
                tc.tile_pool(name="ps_t", bufs=3, space="PSUM"))
            ps_m = ctx.enter_context(
                tc.tile_pool(name="ps_m", bufs=2, space="PSUM"))

            # ---- constants ----
            ident = consts.tile([128, 128], F32)
            make_identity(nc, ident[:])
            wx_sb = consts.tile([128, 2, 64], F32)
            nc.sync.dma_start(out=wx_sb[:],
                              in_=bass.AP(tensor=wx_t, offset=0,
                                          ap=[[64, 128], [8192, 2], [1, 64]]))
            bx_sb = consts.tile([64, 1], F32)
            nc.sync.dma_start(out=bx_sb[:], in_=bx_t[:])
            wv_sb = consts.tile([128, 2, 256], F32)
            nc.sync.dma_start(out=wv_sb[:],
                              in_=bass.AP(tensor=wv_t, offset=0,
                                          ap=[[256, 128], [32768, 2], [1, 256]]))
            wo_sb = consts.tile([128, 2, 256], F32)
            nc.sync.dma_start(out=wo_sb[:],
                              in_=bass.AP(tensor=wo_t, offset=0,
                                          ap=[[256, 128], [32768, 2], [1, 256]]))
            bv_b = consts.tile([128, 256], F32)
            nc.sync.dma_start(out=bv_b[:],
                              in_=bass.AP(tensor=bv_t, offset=0,
                                          ap=[[0, 128], [1, 256]]))
            bo_b = consts.tile([128, 256], F32)
            nc.sync.dma_start(out=bo_b[:],
                              in_=bass.AP(tensor=bo_t, offset=0,
                                          ap=[[0, 128], [1, 256]]))
            qn = consts.tile([128, NT, 256], F32)
            nc.sync.dma_start(out=qn[:],
                              in_=bass.AP(tensor=qs_t, offset=0,
                                          ap=[[256, 128], [32768, NT], [1, 256]]))
            x0i = consts.tile([H * P, NL], I16)
            a_res = consts.tile([128, NT, H * P], F32)
            b_res = consts.tile([128, NT, H * P], F32)

            # ---- loc phase ----
            for t in range(NT):
                qT = locp.tile([128, 2, 128], F32, tag="qT")
                for c in range(2):
                    pt = ps_t.tile([128, 128], F32, tag="pt")
                    nc.tensor.transpose(out=pt[:],
                                        in_=qn[:, t, 128 * c:128 * (c + 1)],
                                        identity=ident[:])
                    nc.scalar.copy(out=qT[:, c, :], in_=pt[:])
                pl = ps_m.tile([64, 128], F32, tag="pm")
                for c in range(2):
                    nc.tensor.matmul(out=pl[:], lhsT=wx_sb[:, c, :], rhs=qT[:, c, :],
                                     start=(c == 0), stop=(c == 1))
                sg = locp.tile([32, 128], F32, tag="sg")
                nc.scalar.activation(out=sg[:], in_=pl[0:32, :], func=AF.Sigmoid,
                                     bias=bx_sb[0:32, 0:1])
                off = locp.tile([32, 128], F32, tag="off")
                nc.scalar.activation(out=off[:], in_=pl[32:64, :],
                                     func=AF.Identity, bias=bx_sb[32:64, 0:1])
                x = locp.tile([32, 128], F32, tag="x")
                nc.vector.tensor_add(out=x[:], in0=sg[:], in1=off[:])
                nc.vector.tensor_scalar_mul(out=x[:], in0=x[:], scalar1=SCALE)
                y = locp.tile([32, 128], F32, tag="y")
                nc.vector.tensor_scalar_sub(out=y[:], in0=x[:], scalar1=0.5)
                yi = locp.tile([32, 128], I32, tag="yi")
                nc.vector.tensor_copy(out=yi[:], in_=y[:])
                x0c = locp.tile([32, 128], F32, tag="x0c")
                nc.vector.tensor_copy(out=x0c[:], in_=yi[:])
                nc.vector.tensor_scalar(out=x0c[:], in0=x0c[:], scalar1=0.0,
                                        scalar2=float(M - 1), op0=OP.max,
                                        op1=OP.min)
                wxt = locp.tile([32, 128], F32, tag="wxt")
                nc.vector.tensor_sub(out=wxt[:], in0=x[:], in1=x0c[:])
                at = locp.tile([32, 128], F32, tag="at")
                nc.vector.tensor_scalar(out=at[:], in0=wxt[:], scalar1=1.0,
                                        scalar2=-R32, op0=OP.subtract,
                                        op1=OP.mult)
                bt = locp.tile([32, 128], F32, tag="bt")
                nc.vector.tensor_scalar_mul(out=bt[:], in0=wxt[:], scalar1=R32)
                nc.vector.tensor_copy(out=x0i[:, 128 * t:128 * (t + 1)],
                                      in_=x0c[:])
                for src, dst in ((at, a_res), (bt, b_res)):
                    pt2 = ps_t.tile([128, 32], F32, tag="pt")
                    nc.tensor.transpose(out=pt2[:], in_=src[:],
                                        identity=ident[0:32, 0:32])
                    nc.vector.tensor_copy(out=dst[:, t, :], in_=pt2[:])
            for h in range(H):
                for ci in range(NCHUNK):
                    nc.scalar.dma_start(
                        out=bass.AP(tensor=x0d,
                                    offset=(h * NCHUNK + ci) * P * QCHUNK,
                                    ap=[[QCHUNK // 16, P], [1, QCHUNK // 16], [P * QCHUNK // 16, 16]]),
                        in_=x0i[4 * h:4 * (h + 1), ci * QCHUNK:(ci + 1) * QCHUNK])

            # ---- vproj + table build ----
            for mt in range(256):
                vtile = vp.tile([128, 256], F32, tag="vtile")
                nc.sync.dma_start(out=vtile[:], in_=val_t[128 * mt:128 * (mt + 1), :])
                ktile = vp.tile([128, 256], F32, tag="ktile")
                nc.scalar.dma_start(out=ktile[:], in_=key_t[128 * mt:128 * (mt + 1), :])
                vT = vp.tile([128, 2, 128], F32, tag="vT")
                for c in range(2):
                    pt = ps_t.tile([128, 128], F32, tag="pt")
                    nc.tensor.transpose(out=pt[:], in_=vtile[:, 128 * c:128 * (c + 1)],
                                        identity=ident[:])
                    nc.scalar.copy(out=vT[:, c, :], in_=pt[:])
                pv = ps_m.tile([128, 256], F32, tag="pm")
                for c in range(2):
                    nc.tensor.matmul(out=pv[:], lhsT=vT[:, c, :], rhs=wv_sb[:, c, :],
                                     start=(c == 0), stop=(c == 1))
                csb = vp.tile([128, H, 2 * CH], F32, tag="csb")
                nc.vector.tensor_add(
                    out=csb[:, :, 0:CH],
                    in0=pv[:].rearrange("p (h e) -> p h e", h=H),
                    in1=bv_b[:].rearrange("p (h e) -> p h e", h=H))
                nc.vector.tensor_copy(
                    out=csb[:, :, CH:2 * CH],
                    in_=ktile[:].rearrange("p (h e) -> p h e", h=H))
                eng = [nc.sync, nc.scalar]
                eng[mt % 2].dma_start(
                    out=bass.AP(tensor=ckv, offset=128 * mt * 2 * CH,
                                ap=[[2 * CH, 128], [(M + 2) * 2 * CH, H], [1, 2 * CH]]),
                    in_=csb[:])
                if mt == 255:
                    eng[1].dma_start(
                        out=bass.AP(tensor=ckv, offset=M * 2 * CH,
                                    ap=[[2 * CH, 1], [(M + 2) * 2 * CH, H], [1, 2 * CH]]),
                        in_=csb[127:128, :, :])

            # ---- main loop: gather + attention ----
            for ci in range(NCHUNK):
                oacc = mp.tile([128, QS, H, CH], F32, tag="oacc")
                for h in range(H):
                    idx = mp.tile([128, P * (QCHUNK // 16)], I16, tag="idx")
                    nc.sync.dma_start(
                        out=idx[:],
                        in_=bass.AP(tensor=x0d,
                                    offset=(h * NCHUNK + ci) * P * QCHUNK,
                                    ap=[[0, 8], [P * QCHUNK // 16, 16], [1, P * QCHUNK // 16]]))
                    g = mp.tile([128, NIDX // 128, 2 * 2 * CH], F32, tag="g")
                    nc.gpsimd.dma_gather(
                        out_ap=g[:],
                        in_ap=bass.AP(tensor=ckv, offset=h * (M + 2) * 2 * CH,
                                      ap=[[2 * CH, M], [1, 4 * CH]]),
                        idxs_ap=idx[:],
                        num_idxs=NIDX,
                        num_idxs_reg=NIDX,
                        elem_size=4 * CH,
                        elem_step=2 * CH,
                        single_packet=False,
                    )
                    g4 = g[:].rearrange("p (a b) e -> p a b e", a=P)
                    qb = _bcast(qn[:, ci * QS:(ci + 1) * QS, CH * h:CH * (h + 1)], 1, P)
                    prod = smp.tile([128, P, QS, CH], F32, tag="prod")
                    s0 = smp.tile([128, P, QS], F32, tag="s0")
                    nc.vector.tensor_mul(out=prod[:], in0=g4[:, :, :, CH:2 * CH], in1=qb)
                    nc.vector.tensor_reduce(out=s0[:], in_=prod[:],
                                            axis=mybir.AxisListType.X, op=OP.add)
                    s1 = smp.tile([128, P, QS], F32, tag="s1")
                    nc.vector.tensor_mul(out=prod[:], in0=g4[:, :, :, 3 * CH:4 * CH], in1=qb)
                    nc.vector.tensor_reduce(out=s1[:], in_=prod[:],
                                            axis=mybir.AxisListType.X, op=OP.add)
                    asl = a_res[:, ci * QS:(ci + 1) * QS, P * h:P * (h + 1)] \
                        .rearrange("p a b -> p b a")
                    bsl = b_res[:, ci * QS:(ci + 1) * QS, P * h:P * (h + 1)] \
                        .rearrange("p a b -> p b a")
                    s = smp.tile([128, P, QS], F32, tag="s")
                    nc.vector.tensor_mul(out=s[:], in0=s0[:], in1=asl)
                    nc.vector.tensor_mul(out=s1[:], in0=s1[:], in1=bsl)
                    nc.vector.tensor_add(out=s[:], in0=s[:], in1=s1[:])
                    # softmax over P
                    mx = smp.tile([128, QS], F32, tag="mx")
                    m2 = smp.tile([128, QS], F32, tag="m2")
                    nc.vector.tensor_tensor(out=mx[:], in0=s[:, 0, :], in1=s[:, 1, :], op=OP.max)
                    nc.vector.tensor_tensor(out=m2[:], in0=s[:, 2, :], in1=s[:, 3, :], op=OP.max)
                    nc.vector.tensor_tensor(out=mx[:], in0=mx[:], in1=m2[:], op=OP.max)
                    mxb = _bcast(mx[:], 1, P)
                    nc.vector.tensor_sub(out=s[:], in0=s[:], in1=mxb)
                    e = smp.tile([128, P, QS], F32, tag="e")
                    nc.scalar.activation(out=e[:], in_=s[:], func=AF.Exp)
                    su = smp.tile([128, QS], F32, tag="su")
                    nc.vector.tensor_add(out=su[:], in0=e[:, 0, :], in1=e[:, 1, :])
                    nc.vector.tensor_add(out=m2[:], in0=e[:, 2, :], in1=e[:, 3, :])
                    nc.vector.tensor_add(out=su[:], in0=su[:], in1=m2[:])
                    rcp = smp.tile([128, QS], F32, tag="rcp")
                    nc.vector.reciprocal(out=rcp[:], in_=su[:])
                    nc.vector.tensor_scalar_mul(out=rcp[:], in0=rcp[:],
                                                scalar1=float(np.sqrt(np.float32(CH))))
                    attn = smp.tile([128, P, QS], F32, tag="attn")
                    rcb = _bcast(rcp[:], 1, P)
                    nc.vector.tensor_mul(out=attn[:], in0=e[:], in1=rcb)
                    a0 = smp.tile([128, P, QS], F32, tag="a0")
                    a1 = smp.tile([128, P, QS], F32, tag="a1")
                    nc.vector.tensor_mul(out=a0[:], in0=attn[:], in1=asl)
                    nc.vector.tensor_mul(out=a1[:], in0=attn[:], in1=bsl)
                    # weighted values
                    t3 = smp.tile([128, P, QS, CH], F32, tag="t3")
                    a0b = _bcast(a0[:], 3, CH)
                    a1b = _bcast(a1[:], 3, CH)
                    nc.vector.tensor_mul(out=t3[:], in0=g4[:, :, :, 0:CH], in1=a0b)
                    nc.vector.tensor_mul(out=prod[:], in0=g4[:, :, :, 2 * CH:3 * CH], in1=a1b)
                    nc.vector.tensor_add(out=t3[:], in0=t3[:], in1=prod[:])
                    u0 = smp.tile([128, QS, CH], F32, tag="u0")
                    nc.vector.tensor_add(out=u0[:], in0=t3[:, 0, :, :], in1=t3[:, 1, :, :])
                    u1 = smp.tile([128, QS, CH], F32, tag="u1")
                    nc.vector.tensor_add(out=u1[:], in0=t3[:, 2, :, :], in1=t3[:, 3, :, :])
                    nc.vector.tensor_add(out=oacc[:, :, h, :], in0=u0[:], in1=u1[:])
                # output projection for this chunk
                for q in range(QS):
                    rT = mp.tile([128, 2, 128], F32, tag="rT")
                    rsl = oacc[:, q, :, :].rearrange("p h e -> p (h e)")
                    for c in range(2):
                        pt = ps_t.tile([128, 128], F32, tag="pt")
                        nc.tensor.transpose(out=pt[:], in_=rsl[:, 128 * c:128 * (c + 1)],
                                            identity=ident[:])
                        nc.scalar.copy(out=rT[:, c, :], in_=pt[:])
                    po = ps_m.tile([128, 256], F32, tag="pm")
                    for c in range(2):
                        nc.tensor.matmul(out=po[:], lhsT=rT[:, c, :], rhs=wo_sb[:, c, :],
                                         start=(c == 0), stop=(c == 1))
                    osb = mp.tile([128, 256], F32, tag="osb")
                    nc.vector.tensor_add(out=osb[:], in0=po[:], in1=bo_b[:])
                    row0 = ci * QCHUNK + q * 128
                    nc.scalar.dma_start(out=out_t[row0:row0 + 128, :], in_=osb[:])

    nc.compile()
    _split_excess_waits(nc)
    return nc


_PROGRAM = None


def _get_program():
    global _PROGRAM
    if _PROGRAM is None:
        _PROGRAM = build_program()
    return _PROGRAM


def kernel(query, key, value, Wr, br, Wo, bo, Wv, bv, Wout, bout):
    query = np.ascontiguousarray(np.asarray(query, dtype=np.float32))
    key = np.ascontiguousarray(np.asarray(key, dtype=np.float32))
    value = np.ascontiguousarray(np.asarray(value, dtype=np.float32))
    Wr = np.asarray(Wr, dtype=np.float32)
    br = np.asarray(br, dtype=np.float32)
    Wo = np.asarray(Wo, dtype=np.float32)
    bo = np.asarray(bo, dtype=np.float32)
    Wv = np.ascontiguousarray(np.asarray(Wv, dtype=np.float32))
    bv = np.asarray(bv, dtype=np.float32)
    Wout = np.ascontiguousarray(np.asarray(Wout, dtype=np.float32))
    bout = np.asarray(bout, dtype=np.float32)

    Wx = np.ascontiguousarray(
        np.concatenate([Wr[:, 0::2], Wo[:, 0::2]], axis=1))
    bx = np.ascontiguousarray(
        np.concatenate([br[0::2], bo[0::2]])[:, None])

    nc = _get_program()
    in_maps = []
    for c in range(NC_CORES):
        in_maps.append({
            "qs": query[c * NL:(c + 1) * NL],
            "key": key,
            "value": value,
            "Wx": Wx,
            "bx": bx,
            "Wv": Wv,
            "bv": bv,
            "Wout": Wout,
            "bout": bout,
        })
    res = run_bass_kernel_spmd(nc, in_maps, list(range(NC_CORES)))
    out = np.concatenate([res.results[c]["out"] for c in range(NC_CORES)], axis=0)
    return out


# revision 13
# speedup vs baseline: 1.4234x; 1.0712x over previous
"""Deformable cross-attention Trainium2 kernel.

Problem (hardcoded): N=32768 queries, M=32768 kv tokens, C=256, H=8 heads,
P=4 points, 1-D bilinear sampling along the token axis.

Sharding: queries split across 8 NeuronCores (4096/core); key/value tables and
weights replicated. No collectives.

Per-core plan:
  1. loc phase (transposed orientation, [hp=32 partitions x q free]):
     x = (sigmoid(q@Wr_x + br_x) + (q@Wo_x + bo_x)) * (M-1)
     x0 = clamp(round_rne(x-0.5), 0, M-1)   (== floor for lerp purposes)
     wx = x - x0;  A = (1-wx)/sqrt(32);  B = wx/sqrt(32)
     x0 -> int16, dumped to DRAM for gather-index replication.
  2. vproj phase: v = value@Wv + bv per 128-row tile (fp32 matmuls with PE
     transposes), then build the combined per-head table
     ckv[h][m] = [v[m, 32h:32h+32] | key[m, 32h:32h+32]]  (64 f32 = 256B rows)
  3. gather phase: per (q-chunk of 1024, head): one dma_gather op with
     num_idxs=4096 (q x P taps), elem_step=64, elem_size=128 -> each tap
     returns [v0|k0|v1|k1] for rows x0, x0+1. Pad row M duplicates row M-1 so
     the clipped x1 case is exact.
  4. scores s = (q.k0)*A + (q.k1)*B, softmax over P, out += a0*v0 + a1*v1,
     then output projection out@Wout + bout (fp32).
"""
import numpy as np
from contextlib import ExitStack

import concourse.bass as bass
import concourse.tile as tile
from concourse import mybir, bacc
from concourse.bass_utils import run_bass_kernel_spmd
from concourse.masks import make_identity
from concourse.vector_clock import ScopedClock

F32 = mybir.dt.float32
I16 = mybir.dt.int16
I32 = mybir.dt.int32
AF = mybir.ActivationFunctionType
OP = mybir.AluOpType

NC_CORES = 8
N, M, C, H, P = 32768, 32768, 256, 8, 4
CH = C // H                      # 32
NL = N // NC_CORES               # 4096 queries per core
NT = NL // 128                   # 32 q-tiles
QCHUNK = 1024                    # queries per gather chunk
NCHUNK = NL // QCHUNK            # 4
NIDX = QCHUNK * P                # 4096 taps per gather op
QS = QCHUNK // 128               # 8 q-subtiles per chunk
R32 = float(1.0 / np.sqrt(np.float32(CH)))
SCALE = float(M - 1)

# ---------------------------------------------------------------------------
# walrus in this toolchain rejects >1 sem wait per instruction; redistribute
# excess waits onto single-wait no-ops, and do the same for the TileContext
# tail drain.

def _patched_drain_and_barrier(self, tick_clock, wait_clock):
    nc = self.nc
    tmp = nc.sync.nop(nofuse=True)
    wait_clock.add_sem_waits(tmp.ins, ScopedClock({None: tick_clock.global_clock}))
    si = tmp.ins.sync_info
    if si is not None and si.on_wait is not None and len(si.on_wait) > 1:
        waits = list(si.on_wait)
        ups = list(si.on_update or [])
        tmp.ins.sync_info = mybir.SyncInfo(on_wait=waits[:1], on_update=ups)
        for w in waits[1:]:
            n = nc.sync.nop(nofuse=True)
            n.ins.sync_info = mybir.SyncInfo(on_wait=[w], on_update=[])
    nc.sync.drain()
    nc.all_engine_barrier()
    assert self.sems is not None
    popped = nc._tile_sem_poison_stack.pop()
    assert popped is self._sem_poison
    nc.clear_and_free_semaphores(list(self.sems.allocated().values()))
    nc.all_engine_barrier()


def _split_excess_waits(nc, limit=1):
    k = 0
    for f in nc.m.functions:
        for b in f.blocks:
            insts = b.instructions
            out = []
            for ins in insts:
                si = ins.sync_info
                waits = list(si.on_wait) if si is not None and si.on_wait else []
                if len(waits) > limit:
                    rest, keep = waits[:-limit], waits[-limit:]
                    for i in range(0, len(rest), limit):
                        nop = mybir.InstNoOp(name=f"wsplit-{k}", ins=[], outs=[])
                        k += 1
                        nop.engine = ins.engine
                        nop.sync_info = mybir.SyncInfo(
                            on_wait=rest[i:i + limit], on_update=[])
                        try:
                            nc.register_instruction(nop, overwrite=True)
                        except Exception:
                            pass
                        out.append(nop)
                    ins.sync_info = mybir.SyncInfo(
                        on_wait=keep, on_update=list(si.on_update or []))
                out.append(ins)
            if len(out) != len(insts):
                b.instructions = out


tile.TileContext._drain_and_barrier = _patched_drain_and_barrier

# ---------------------------------------------------------------------------


def _bcast(ap, axis, n):
    """Insert a [0, n] broadcast dim at position `axis` of an AP."""
    dims = [list(d) for d in ap.ap]
    dims.insert(axis, [0, n])
    return bass.AP(tensor=ap.tensor, offset=ap.offset, ap=dims)

def build_program():
    nc = bacc.Bacc("TRN2", target_bir_lowering=False, debug=False,
                   num_devices=NC_CORES)

    qs_t = nc.declare_dram_parameter("qs", [NL, C], F32, isOutput=False)
    key_t = nc.declare_dram_parameter("key", [M, C], F32, isOutput=False)
    val_t = nc.declare_dram_parameter("value", [M, C], F32, isOutput=False)
    wx_t = nc.declare_dram_parameter("Wx", [C, 64], F32, isOutput=False)
    bx_t = nc.declare_dram_parameter("bx", [64, 1], F32, isOutput=False)
    wv_t = nc.declare_dram_parameter("Wv", [C, C], F32, isOutput=False)
    bv_t = nc.declare_dram_parameter("bv", [C], F32, isOutput=False)
    wo_t = nc.declare_dram_parameter("Wout", [C, C], F32, isOutput=False)
    bo_t = nc.declare_dram_parameter("bout", [C], F32, isOutput=False)
    out_t = nc.declare_dram_parameter("out", [NL, C], F32, isOutput=True)

    ckv = nc.dram_tensor("ckv", [H, M + 2, 2 * CH], F32)
    x0d = nc.dram_tensor("x0d", [H, NCHUNK, 16, P * QCHUNK // 16], I16)

    with tile.TileContext(nc) as tc:
        with ExitStack() as ctx:
            consts = ctx.enter_context(tc.tile_pool(name="consts", bufs=1))
            locp = ctx.enter_context(tc.tile_pool(name="locp", bufs=2))
            vp = ctx.enter_context(tc.tile_pool(name="vp", bufs=5))
            mp = ctx.enter_context(tc.tile_pool(name="mp", bufs=3))
            smp = ctx.enter_context(tc.tile_pool(name="smp", bufs=2))
            ps_t = ctx.enter_context(
                tc.tile_pool(name="ps_t", bufs=3, space="PSUM"))
            ps_m = ctx.enter_context(
                tc.tile_pool(name="ps_m", bufs=2, space="PSUM"))

            # ---- constants ----
            ident = consts.tile([128, 128], F32)
            make_identity(nc, ident[:])
            wx_sb = consts.tile([128, 2, 64], F32)
            nc.sync.dma_start(out=wx_sb[:],
                              in_=bass.AP(tensor=wx_t, offset=0,
                                          ap=[[64, 128], [8192, 2], [1, 64]]))
            bx_sb = consts.tile([64, 1], F32)
            nc.sync.dma_start(out=bx_sb[:], in_=bx_t[:])
            wv_sb = consts.tile([128, 2, 256], F32)
            nc.sync.dma_start(out=wv_sb[:],
                              in_=bass.AP(tensor=wv_t, offset=0,
                                          ap=[[256, 128], [32768, 2], [1, 256]]))
            wo_sb = consts.tile([128, 2, 256], F32)
            nc.sync.dma_start(out=wo_sb[:],
                              in_=bass.AP(tensor=wo_t, offset=0,
                                          ap=[[256, 128], [32768, 2], [1, 256]]))
            bv_b = consts.tile([128, 256], F32)
            nc.sync.dma_start(out=bv_b[:],
                              in_=bass.AP(tensor=bv_t, offset=0,
                                          ap=[[0, 128], [1, 256]]))
            bo_b = consts.tile([128, 256], F32)
            nc.sync.dma_start(out=bo_b[:],
                              in_=bass.AP(tensor=bo_t, offset=0,
                                          ap=[[0, 128], [1, 256]]))
            qn = consts.tile([128, NT, 256], F32)
            nc.sync.dma_start(out=qn[:],
                              in_=bass.AP(tensor=qs_t, offset=0,
                                          ap=[[256, 128], [32768, NT], [1, 256]]))
            x0i = consts.tile([H * P, NL], I16)
            a_res = consts.tile([128, NT, H * P], F32)
            b_res = consts.tile([128, NT, H * P], F32)

            # ---- loc phase ----
            for t in range(NT):
                qT = locp.tile([128, 2, 128], F32, tag="qT")
                for c in range(2):
                    pt = ps_t.tile([128, 128], F32, tag="pt")
                    nc.tensor.transpose(out=pt[:],
                                        in_=qn[:, t, 128 * c:128 * (c + 1)],
                                        identity=ident[:])
                    nc.scalar.copy(out=qT[:, c, :], in_=pt[:])
                pl = ps_m.tile([64, 128], F32, tag="pm")
                for c in range(2):
                    nc.tensor.matmul(out=pl[:], lhsT=wx_sb[:, c, :], rhs=qT[:, c, :],
                                     start=(c == 0), stop=(c == 1))
                sg = locp.tile([32, 128], F32, tag="sg")
                nc.scalar.activation(out=sg[:], in_=pl[0:32, :], func=AF.Sigmoid,
                                     bias=bx_sb[0:32, 0:1])
                off = locp.tile([32, 128], F32, tag="off")
                nc.scalar.activation(out=off[:], in_=pl[32:64, :],
                                     func=AF.Identity, bias=bx_sb[32:64, 0:1])
                x = locp.tile([32, 128], F32, tag="x")
                nc.vector.tensor_add(out=x[:], in0=sg[:], in1=off[:])
                nc.vector.tensor_scalar_mul(out=x[:], in0=x[:], scalar1=SCALE)
                y = locp.tile([32, 128], F32, tag="y")
                nc.vector.tensor_scalar_sub(out=y[:], in0=x[:], scalar1=0.5)
                yi = locp.tile([32, 128], I32, tag="yi")
                nc.vector.tensor_copy(out=yi[:], in_=y[:])
                x0c = locp.tile([32, 128], F32, tag="x0c")
                nc.vector.tensor_copy(out=x0c[:], in_=yi[:])
                nc.vector.tensor_scalar(out=x0c[:], in0=x0c[:], scalar1=0.0,
                                        scalar2=float(M - 1), op0=OP.max,
                                        op1=OP.min)
                wxt = locp.tile([32, 128], F32, tag="wxt")
                nc.vector.tensor_sub(out=wxt[:], in0=x[:], in1=x0c[:])
                at = locp.tile([32, 128], F32, tag="at")
                nc.vector.tensor_scalar(out=at[:], in0=wxt[:], scalar1=1.0,
                                        scalar2=-R32, op0=OP.subtract,
                                        op1=OP.mult)
                bt = locp.tile([32, 128], F32, tag="bt")
                nc.vector.tensor_scalar_mul(out=bt[:], in0=wxt[:], scalar1=R32)
                nc.vector.tensor_copy(out=x0i[:, 128 * t:128 * (t + 1)],
                                      in_=x0c[:])
                for src, dst in ((at, a_res), (bt, b_res)):
                    pt2 = ps_t.tile([128, 32], F32, tag="pt")
                    nc.tensor.transpose(out=pt2[:], in_=src[:],
                                        identity=ident[0:32, 0:32])
                    nc.vector.tensor_copy(out=dst[:, t, :], in_=pt2[:])
            for h in range(H):
                for ci in range(NCHUNK):
                    nc.scalar.dma_start(
                        out=bass.AP(tensor=x0d,
                                    offset=(h * NCHUNK + ci) * P * QCHUNK,
                                    ap=[[QCHUNK // 16, P], [1, QCHUNK // 16], [P * QCHUNK // 16, 16]]),
                        in_=x0i[4 * h:4 * (h + 1), ci * QCHUNK:(ci + 1) * QCHUNK])

            # ---- vproj + table build ----
            for mt in range(256):
                vtile = vp.tile([128, 256], F32, tag="vtile")
                nc.sync.dma_start(out=vtile[:], in_=val_t[128 * mt:128 * (mt + 1), :])
                ktile = vp.tile([128, 256], F32, tag="ktile")
                nc.scalar.dma_start(out=ktile[:], in_=key_t[128 * mt:128 * (mt + 1), :])
                vT = vp.tile([128, 2, 128], F32, tag="vT")
                for c in range(2):
                    pt = ps_t.tile([128, 128], F32, tag="pt")
                    nc.tensor.transpose(out=pt[:], in_=vtile[:, 128 * c:128 * (c + 1)],
                                        identity=ident[:])
                    nc.scalar.copy(out=vT[:, c, :], in_=pt[:])
                pv = ps_m.tile([128, 256], F32, tag="pm")
                for c in range(2):
                    nc.tensor.matmul(out=pv[:], lhsT=vT[:, c, :], rhs=wv_sb[:, c, :],
                                     start=(c == 0), stop=(c == 1))
                csb = vp.tile([128, H, 2 * CH], F32, tag="csb")
                nc.vector.tensor_add(
                    out=csb[:, :, 0:CH],
                    in0=pv[:].rearrange("p (h e) -> p h e", h=H),
                    in1=bv_b[:].rearrange("p (h e) -> p h e", h=H))
                nc.vector.tensor_copy(
                    out=csb[:, :, CH:2 * CH],
                    in_=ktile[:].rearrange("p (h e) -> p h e", h=H))
                eng = [nc.sync, nc.scalar]
                eng[mt % 2].dma_start(
                    out=bass.AP(tensor=ckv, offset=128 * mt * 2 * CH,
                                ap=[[2 * CH, 128], [(M + 2) * 2 * CH, H], [1, 2 * CH]]),
                    in_=csb[:])
                if mt == 255:
                    eng[1].dma_start(
                        out=bass.AP(tensor=ckv, offset=M * 2 * CH,
                                    ap=[[2 * CH, 1], [(M + 2) * 2 * CH, H], [1, 2 * CH]]),
                        in_=csb[127:128, :, :])

            # ---- main loop: gather + attention ----
            for ci in range(NCHUNK):
                oacc = mp.tile([128, QS, H, CH], F32, tag="oacc")
                for h in range(H):
                    idx = mp.tile([128, P * (QCHUNK // 16)], I16, tag="idx")
                    nc.sync.dma_start(
                        out=idx[:],
                        in_=bass.AP(tensor=x0d,
                                    offset=(h * NCHUNK + ci) * P * QCHUNK,
                                    ap=[[0, 8], [P * QCHUNK // 16, 16], [1, P * QCHUNK // 16]]))
                    g = mp.tile([128, NIDX // 128, 2 * 2 * CH], F32, tag="g")
                    nc.gpsimd.dma_gather(
                        out_ap=g[:],
                        in_ap=bass.AP(tensor=ckv, offset=h * (M + 2) * 2 * CH,
                                      ap=[[2 * CH, M], [1, 4 * CH]]),
                        idxs_ap=idx[:],
                        num_idxs=NIDX,
                        num_idxs_reg=NIDX,
                        elem_size=4 * CH,
                        elem_step=2 * CH,
                        single_packet=False,
                    )
                    g4 = g[:].rearrange("p (a b) e -> p a b e", a=P)
                    qb = _bcast(qn[:, ci * QS:(ci + 1) * QS, CH * h:CH * (h + 1)], 1, P)
                    prod = smp.tile([128, P, QS, CH], F32, tag="prod")
                    s0 = smp.tile([128, P, QS], F32, tag="s0")
                    nc.vector.tensor_mul(out=prod[:], in0=g4[:, :, :, CH:2 * CH], in1=qb)
                    nc.vector.tensor_reduce(out=s0[:], in_=prod[:],
                                            axis=mybir.AxisListType.X, op=OP.add)
                    s1 = smp.tile([128, P, QS], F32, tag="s1")
                    nc.vector.tensor_mul(out=prod[:], in0=g4[:, :, :, 3 * CH:4 * CH], in1=qb)
                    nc.vector.tensor_reduce(out=s1[:], in_=prod[:],
                                            axis=mybir.AxisListType.X, op=OP.add)
                    asl = a_res[:, ci * QS:(ci + 1) * QS, P * h:P * (h + 1)] \
                        .rearrange("p a b -> p b a")
                    bsl = b_res[:, ci * QS:(ci + 1) * QS, P * h:P * (h + 1)] \
                        .rearrange("p a b -> p b a")
                    s = smp.tile([128, P, QS], F32, tag="s")
                    nc.vector.tensor_mul(out=s[:], in0=s0[:], in1=asl)
                    nc.vector.tensor_mul(out=s1[:], in0=s1[:], in1=bsl)
                    nc.vector.tensor_add(out=s[:], in0=s[:], in1=s1[:])
                    # softmax over P
                    mx = smp.tile([128, QS], F32, tag="mx")
                    m2 = smp.tile([128, QS], F32, tag="m2")
                    nc.vector.tensor_tensor(out=mx[:], in0=s[:, 0, :], in1=s[:, 1, :], op=OP.max)
                    nc.vector.tensor_tensor(out=m2[:], in0=s[:, 2, :], in1=s[:, 3, :], op=OP.max)
                    nc.vector.tensor_tensor(out=mx[:], in0=mx[:], in1=m2[:], op=OP.max)
                    mxb = _bcast(mx[:], 1, P)
                    nc.vector.tensor_sub(out=s[:], in0=s[:], in1=mxb)
                    e = smp.tile([128, P, QS], F32, tag="e")
                    nc.scalar.activation(out=e[:], in_=s[:], func=AF.Exp)
                    su = smp.tile([128, QS], F32, tag="su")
                    nc.vector.tensor_add(out=su[:], in0=e[:, 0, :], in1=e[:, 1, :])
                    nc.vector.tensor_add(out=m2[:], in0=e[:, 2, :], in1=e[:, 3, :])
                    nc.vector.tensor_add(out=su[:], in0=su[:], in1=m2[:])
                    rcp = smp.tile([128, QS], F32, tag="rcp")
                    nc.vector.reciprocal(out=rcp[:], in_=su[:])
                    nc.vector.tensor_scalar_mul(out=rcp[:], in0=rcp[:],
                                                scalar1=float(np.sqrt(np.float32(CH))))
                    attn = smp.tile([128, P, QS], F32, tag="attn")
                    rcb = _bcast(rcp[:], 1, P)
                    nc.vector.tensor_mul(out=attn[:], in0=e[:], in1=rcb)
                    a0 = smp.tile([128, P, QS], F32, tag="a0")
                    a1 = smp.tile([128, P, QS], F32, tag="a1")
                    nc.vector.tensor_mul(out=a0[:], in0=attn[:], in1=asl)
                    nc.vector.tensor_mul(out=a1[:], in0=attn[:], in1=bsl)
                    # weighted values
                    t3 = smp.tile([128, P, QS, CH], F32, tag="t3")
                    a0b = _bcast(a0[:], 3, CH)
                    a1b = _bcast(a1[:], 3, CH)
                    nc.vector.tensor_mul(out=t3[:], in0=g4[:, :, :, 0:CH], in1=a0b)
                    nc.vector.tensor_mul(out=prod[:], in0=g4[:, :, :, 2 * CH:3 * CH], in1=a1b)
                    nc.vector.tensor_add(out=t3[:], in0=t3[:], in1=prod[:])
                    u0 = smp.tile([128, QS, CH], F32, tag="u0")
                    nc.vector.tensor_add(out=u0[:], in0=t3[:, 0, :, :], in1=t3[:, 1, :, :])
                    u1 = smp.tile([128, QS, CH], F32, tag="u1")
                    nc.vector.tensor_add(out=u1[:], in0=t3[:, 2, :, :], in1=t3[:, 3, :, :])
                    nc.vector.tensor_add(out=oacc[:, :, h, :], in0=u0[:], in1=u1[:])
                # output projection for this chunk
                for q in range(QS):
                    rT = mp.tile([128, 2, 128], F32, tag="rT")
                    rsl = oacc[:, q, :, :].rearrange("p h e -> p (h e)")
                    for c in range(2):
                        pt = ps_t.tile([128, 128], F32, tag="pt")
                        nc.tensor.transpose(out=pt[:], in_=rsl[:, 128 * c:128 * (c + 1)],
                                            identity=ident[:])
                        nc.scalar.copy(out=rT[:, c, :], in_=pt[:])
                    po = ps_m.tile([128, 256], F32, tag="pm")
                    for c in range(2):
                        nc.tensor.matmul(out=po[:], lhsT=rT[:, c, :], rhs=wo_sb[:, c, :],
                                         start=(c == 0), stop=(c == 1))
                    osb = mp.tile([128, 256], F32, tag="osb")
                    nc.vector.tensor_add(out=osb[:], in0=po[:], in1=bo_b[:])
                    row0 = ci * QCHUNK + q * 128
                    nc.scalar.dma_start(out=out_t[row0:row0 + 128, :], in_=osb[:])

    nc.compile()
    _split_excess_waits(nc)
    return nc


_PROGRAM = None


def _get_program():
    global _PROGRAM
    if _PROGRAM is None:
        _PROGRAM = build_program()
    return _PROGRAM


def kernel(query, key, value, Wr, br, Wo, bo, Wv, bv, Wout, bout):
    query = np.ascontiguousarray(np.asarray(query, dtype=np.float32))
    key = np.ascontiguousarray(np.asarray(key, dtype=np.float32))
    value = np.ascontiguousarray(np.asarray(value, dtype=np.float32))
    Wr = np.asarray(Wr, dtype=np.float32)
    br = np.asarray(br, dtype=np.float32)
    Wo = np.asarray(Wo, dtype=np.float32)
    bo = np.asarray(bo, dtype=np.float32)
    Wv = np.ascontiguousarray(np.asarray(Wv, dtype=np.float32))
    bv = np.asarray(bv, dtype=np.float32)
    Wout = np.ascontiguousarray(np.asarray(Wout, dtype=np.float32))
    bout = np.asarray(bout, dtype=np.float32)

    Wx = np.ascontiguousarray(
        np.concatenate([Wr[:, 0::2], Wo[:, 0::2]], axis=1))
    bx = np.ascontiguousarray(
        np.concatenate([br[0::2], bo[0::2]])[:, None])

    nc = _get_program()
    in_maps = []
    for c in range(NC_CORES):
        in_maps.append({
            "qs": query[c * NL:(c + 1) * NL],
            "key": key,
            "value": value,
            "Wx": Wx,
            "bx": bx,
            "Wv": Wv,
            "bv": bv,
            "Wout": Wout,
            "bout": bout,
        })
    res = run_bass_kernel_spmd(nc, in_maps, list(range(NC_CORES)))
    out = np.concatenate([res.results[c]["out"] for c in range(NC_CORES)], axis=0)
    return out
